# revision 1
# baseline (speedup 1.0000x reference)
"""AttnPool1D Trainium2 kernel.

out[b, d] = sum_t softmax_t(q . x[b,t,:] / sqrt(D), masked) * x[b,t,d]

Strategy (data-parallel over batch, 4 batches per core, 8 cores):
  - Stream x through SBUF once in 4MB chunks (1024 tokens).
  - Scores: fused multiply+reduce on DVE (scalar_tensor_tensor with
    accum_out), against a host-replicated q/sqrt(D) tile.
  - Mask: host-precomputed additive -1e30, added before Exp on ACT.
  - No max-subtraction needed: scores are O(1) by construction
    (q ~ N(0, 1/D) per element -> scores have std ~ 1/sqrt(D)).
  - Pooling: PE matmul (u^T @ x_tile) accumulated in PSUM over all 32
    token tiles of a batch; partition reduction is free via the matmul.
  - Normalization: L = ones-matmul of per-partition sums; multiply by
    1/L on DVE; DMA the (1, 1024) row out.
"""
import math

import numpy as np

import concourse.tile as tile
from concourse import bacc, mybir
from concourse.bass_utils import run_bass_kernel_spmd

B, T, D = 32, 4096, 1024
NCORES = 8
BPC = B // NCORES       # batches per core
P = 128                 # SBUF partitions / tokens per tile
JT = T // P             # 32 token-tiles per batch
CT = 8                  # token-tiles per chunk (4MB DMA)
NCH = JT // CT          # 4 chunks per batch
MASK_NEG = -1.0e30

F32 = mybir.dt.float32
F32R = mybir.dt.float32r


def build(use_f32r: bool):
    nc = bacc.Bacc("TRN2", target_bir_lowering=False, debug=False)
    x = nc.dram_tensor("x", [BPC, T, D], F32, kind="ExternalInput")
    q = nc.dram_tensor("q128", [P, D], F32, kind="ExternalInput")
    md = nc.dram_tensor("madd", [BPC, P, JT], F32, kind="ExternalInput")
    out = nc.dram_tensor("out", [BPC, D], F32, kind="ExternalOutput")

    with tile.TileContext(nc) as tc:
        with (
            tc.tile_pool(name="const", bufs=1) as constp,
            tc.tile_pool(name="xch", bufs=3) as xp,
            tc.tile_pool(name="bt", bufs=2) as bp,
            tc.tile_pool(name="sm", bufs=2) as sp,
            tc.tile_pool(name="ps", bufs=2, space="PSUM") as pp,
        ):
            qt = constp.tile([P, D], F32)
            nc.sync.dma_start(qt[:], q[:])
            ones = constp.tile([P, 1], F32)
            nc.vector.memset(ones[:], 1.0)
            dummy = constp.tile([P, 1], F32)

            for b in range(BPC):
                mdt = bp.tile([P, JT], F32, tag="mdt")
                nc.sync.dma_start(mdt[:], md[b])
                st = bp.tile([P, JT], F32, tag="st")
                ut = bp.tile([P, JT], F32, tag="ut")
                if use_f32r:
                    utr = bp.tile([P, JT], F32R, tag="utr")
                ps0 = pp.tile([1, 512], F32, tag="ps0")
                ps1 = pp.tile([1, 512], F32, tag="ps1")
                psl = pp.tile([1, 1], F32, tag="psl")

                for c in range(NCH):
                    xc = xp.tile([P, CT * D], F32, tag="xc")
                    nc.sync.dma_start(
                        xc[:].rearrange("p (j d) -> p j d", d=D),
                        x[b, c * CT * P:(c + 1) * CT * P, :].rearrange(
                            "(j p) d -> p j d", p=P
                        ),
                    )
                    if use_f32r:
                        xcr = xp.tile([P, CT * D], F32R, tag="xcr")
                        nc.scalar.activation(
                            xcr[:], xc[:], mybir.ActivationFunctionType.Copy
                        )
                    # scores: st[:, jj] = sum_d x_tile * q
                    for j in range(CT):
                        jj = c * CT + j
                        nc.vector.scalar_tensor_tensor(
                            out=dummy[:].broadcast_to((P, D)),
                            in0=xc[:, j * D:(j + 1) * D],
                            scalar=1.0,
                            in1=qt[:],
                            op0=mybir.AluOpType.mult,
                            op1=mybir.AluOpType.mult,
                            accum_out=st[:, jj:jj + 1],
                        )
                    sl = slice(c * CT, (c + 1) * CT)
                    nc.vector.tensor_add(st[:, sl], st[:, sl], mdt[:, sl])
                    nc.scalar.activation(
                        ut[:, sl], st[:, sl], mybir.ActivationFunctionType.Exp
                    )
                    if use_f32r:
                        nc.vector.tensor_copy(utr[:, sl], ut[:, sl])
                    xsrc = xcr if use_f32r else xc
                    usrc = utr if use_f32r else ut
                    # pooling: psum(1, 1024) += u^T @ x_tile
                    for j in range(CT):
                        jj = c * CT + j
                        ucol = usrc[:, jj:jj + 1]
                        nc.tensor.matmul(
                            ps0[:], ucol, xsrc[:, j * D:j * D + 512],
                            start=(jj == 0), stop=(jj == JT - 1),
                        )
                        nc.tensor.matmul(
                            ps1[:], ucol, xsrc[:, j * D + 512:(j + 1) * D],
                            start=(jj == 0), stop=(jj == JT - 1),
                        )

                # epilogue: L = sum(u); out_row = psum / L
                lsum = sp.tile([P, 1], F32, tag="lsum")
                nc.vector.reduce_sum(lsum[:], ut[:], axis=mybir.AxisListType.X)
                nc.tensor.matmul(psl[:], lsum[:], ones[:], start=True, stop=True)
                linv = sp.tile([1, 1], F32, tag="linv")
                nc.vector.reciprocal(linv[:], psl[:])
                orow = sp.tile([1, D], F32, tag="orow")
                nc.vector.tensor_scalar_mul(orow[:, 0:512], ps0[:], linv[:])
                nc.vector.tensor_scalar_mul(orow[:, 512:1024], ps1[:], linv[:])
                nc.sync.dma_start(out[b:b + 1, :], orow[:])

    nc.compile()
    return nc


def prepare_in_maps(x, mask, query):
    xs = np.ascontiguousarray(x, dtype=np.float32).reshape(NCORES, BPC, T, D)
    q128 = np.ascontiguousarray(
        np.broadcast_to(
            (np.asarray(query, dtype=np.float32)[0, 0] / math.sqrt(D)), (P, D)
        )
    )
    madd = np.where(np.asarray(mask, dtype=bool), np.float32(MASK_NEG), np.float32(0.0))
    madd = madd.astype(np.float32).reshape(B, JT, P).transpose(0, 2, 1)
    madd = np.ascontiguousarray(madd).reshape(NCORES, BPC, P, JT)
    return [
        {"x": xs[i], "q128": q128, "madd": madd[i]} for i in range(NCORES)
    ]


def run(x, mask, query, use_f32r=False, trace=False):
    nc = build(use_f32r)
    res = run_bass_kernel_spmd(
        nc, prepare_in_maps(x, mask, query), list(range(NCORES)), trace=trace
    )
    out = np.concatenate(
        [res.results[i]["out"] for i in range(NCORES)], axis=0
    ).astype(np.float32)
    assert out.shape == (B, D)
    return out, res


def kernel(x, mask, query):
    out, _ = run(x, mask, query, use_f32r=False, trace=False)
    return out
